# revision 3
# baseline (speedup 1.0000x reference)
"""Trainium2 Bass kernel for nn_AbsoluteAttention (B=2,T=2048,DIM=1024,H=16,DH=64,DT=32).

Key algebraic restructuring (exact in real arithmetic):
  - sum(softmax(q), -1) == 1, so the whole Q path collapses:
    q_attentioned[b,t,l,h] = sum_d t_q[t,h,d] * t_k[l,h,d]  (rank-64, data-independent)
  - loading[b,:,h,:] = t_q[:,h,:] @ (t_k[:,h,:].T @ kv[b,:,h,:])  (associativity)
    -> no [T,T] attention matrix is ever materialized.
  - RMSNorm scale rnorm[t] is folded into the softmax logits scale (ACT 'scale')
    and the kv normalization (1/s * rnorm), ln_w folded into Wk/Wv on host.

Sharding: 8 cores = (batch b in {0,1}) x (head-group hg in {0..3}, 4 heads each).
Each core computes a partial output [T, DIM] = loading_hg @ Wo_hg.T for its batch;
host sums the 4 head-group partials per batch.  No collectives.
"""
import numpy as np

try:
    import concourse.bass as bass  # noqa
except ImportError:
    import sys
    for p in ('/opt/trn_rl_repo', '/root/.axon_site/_ro/trn_rl_repo'):
        sys.path.append(p)

import concourse.bacc as bacc
import concourse.mybir as mybir
from concourse.tile import TileContext
from concourse import bass_utils

B, T, DIM, H, DH, DT = 2, 2048, 1024, 16, 64, 32
EPS = float(np.finfo(np.float32).eps)
INV_SQRT_DH = np.float32(1.0 / np.sqrt(DH))
HG = 4            # head-groups
HPG = H // HG     # heads per group (4)
JG = HPG * DH     # j-dims per group (256)
NT = T // 128     # 16 token tiles
F32 = mybir.dt.float32
F32R = mybir.dt.float32r
EXP = mybir.ActivationFunctionType.Exp
AX = mybir.AxisListType.X


def build_nc(niter=1):
    nc = bacc.Bacc("TRN2", target_bir_lowering=False, debug=False)
    sT_d = nc.dram_tensor("sT", [DIM, T], F32R, kind="ExternalInput").ap()
    wkT_d = nc.dram_tensor("wkT", [DIM, JG], F32R, kind="ExternalInput").ap()
    wvT_d = nc.dram_tensor("wvT", [DIM, JG], F32R, kind="ExternalInput").ap()
    woT_d = nc.dram_tensor("woT", [JG, DIM], F32R, kind="ExternalInput").ap()
    tqT_d = nc.dram_tensor("tqT", [JG, T], F32R, kind="ExternalInput").ap()
    tk_d = nc.dram_tensor("tk", [T, 2 * DT], F32, kind="ExternalInput").ap()
    sc_d = nc.dram_tensor("sc", [T], F32, kind="ExternalInput").ap()
    rn_d = nc.dram_tensor("rn", [T], F32, kind="ExternalInput").ap()
    out_d = nc.dram_tensor("out", [T, DIM], F32, kind="ExternalOutput").ap()

    with TileContext(nc) as tc:
        with tc.tile_pool(name="singles", bufs=1) as singles, \
             tc.tile_pool(name="stp", bufs=2) as stp, \
             tc.tile_pool(name="work", bufs=3) as work, \
             tc.tile_pool(name="small", bufs=4) as small, \
             tc.tile_pool(name="outp", bufs=2) as outp, \
             tc.tile_pool(name="ppk", bufs=2, space="PSUM") as ppk, \
             tc.tile_pool(name="ppv", bufs=2, space="PSUM") as ppv, \
             tc.tile_pool(name="ppS", bufs=1, space="PSUM") as ppS, \
             tc.tile_pool(name="pp3", bufs=2, space="PSUM") as pp3:

            # ---- weights / constants (loaded once) ----
            wk_s = singles.tile([128, 8, JG], F32R)
            nc.sync.dma_start(out=wk_s, in_=wkT_d.rearrange("(c p) n -> p c n", p=128))
            wv_s = singles.tile([128, 8, JG], F32R)
            nc.sync.dma_start(out=wv_s, in_=wvT_d.rearrange("(c p) n -> p c n", p=128))
            wo_s = singles.tile([128, 2, DIM], F32R)
            nc.sync.dma_start(out=wo_s, in_=woT_d.rearrange("(c p) n -> p c n", p=128))
            tq_s = singles.tile([64, HPG, T], F32R)
            nc.sync.dma_start(out=tq_s, in_=tqT_d.rearrange("(h p) n -> p h n", p=64))
            tk_s = singles.tile([128, NT, 2 * DT], F32)
            nc.sync.dma_start(out=tk_s, in_=tk_d.rearrange("(n p) d -> p n d", p=128))
            sc_s = singles.tile([128, NT], F32)
            nc.sync.dma_start(out=sc_s, in_=sc_d.rearrange("(n p) -> p n", p=128))
            rn_s = singles.tile([128, NT], F32)
            nc.sync.dma_start(out=rn_s, in_=rn_d.rearrange("(n p) -> p n", p=128))

            for it in range(niter):
                # ---- phase 1: projections + softmax + kv + S accumulation ----
                ps_S = ppS.tile([64, JG], F32, name=f"ps_S_{it}", tag="ps_S")
                for g in range(4):          # 4 groups of 512 tokens
                    st_g = stp.tile([128, 8, 512], F32R, name=f"st_{it}_{g}", tag="st")
                    for c in range(8):
                        nc.sync.dma_start(
                            out=st_g[:, c, :],
                            in_=sT_d[c * 128:(c + 1) * 128, g * 512:(g + 1) * 512])
                    for il in range(4):
                        i = g * 4 + il
                        tsl = slice(il * 128, (il + 1) * 128)
                        psk = ppk.tile([128, JG], F32, name=f"psk_{it}_{i}", tag="psk")
                        psv = ppv.tile([128, JG], F32, name=f"psv_{it}_{i}", tag="psv")
                        for c in range(8):
                            nc.tensor.matmul(psk, st_g[:, c, tsl], wk_s[:, c, :],
                                             start=(c == 0), stop=(c == 7))
                        for c in range(8):
                            nc.tensor.matmul(psv, st_g[:, c, tsl], wv_s[:, c, :],
                                             start=(c == 0), stop=(c == 7))
                        e_t = work.tile([128, JG], F32, tag="e")
                        nc.scalar.activation(out=e_t, in_=psk, func=EXP,
                                             scale=sc_s[:, i:i + 1])
                        ssum = small.tile([128, HPG], F32, tag="ssum")
                        nc.vector.reduce_sum(
                            out=ssum.rearrange("p (f o) -> p f o", o=1),
                            in_=e_t.rearrange("p (h d) -> p h d", h=HPG), axis=AX)
                        ev_t = work.tile([128, JG], F32, tag="ev")
                        nc.vector.tensor_mul(ev_t, e_t, psv)
                        rec = small.tile([128, HPG], F32, tag="rec")
                        nc.vector.reciprocal(rec, ssum)
                        rfin = small.tile([128, HPG], F32, tag="rfin")
                        nc.vector.tensor_scalar_mul(rfin, rec, rn_s[:, i:i + 1])
                        tksc = work.tile([128, HPG, 2 * DT], F32, tag="tksc")
                        for h in range(HPG):
                            nc.vector.tensor_scalar_mul(
                                tksc[:, h, :], tk_s[:, i, :], rfin[:, h:h + 1])
                        for h in range(HPG):
                            nc.tensor.matmul(
                                ps_S[:, h * DH:(h + 1) * DH],
                                tksc[:, h, :], ev_t[:, h * DH:(h + 1) * DH],
                                start=(i == 0 and h == 0),
                                stop=(i == NT - 1 and h == HPG - 1))

                # ---- phase 2: loading_T = S^T-style matmul into lt_s ----
                S_sb = singles.tile([64, HPG, DH], F32R, name=f"S_sb_{it}", tag="S_sb")
                nc.vector.tensor_copy(S_sb.rearrange("p h d -> p (h d)"), ps_S)
                lt_s = singles.tile([128, 2, T], F32R, name=f"lt_{it}", tag="lt")
                for h in range(HPG):
                    for q in range(4):      # T/512 chunks
                        psl = pp3.tile([64, 512], F32, name=f"psl_{it}_{h}_{q}",
                                       tag="p3")
                        nc.tensor.matmul(psl, S_sb[:, h, :],
                                         tq_s[:, h, q * 512:(q + 1) * 512],
                                         start=True, stop=True)
                        nc.vector.tensor_copy(
                            lt_s[(h % 2) * 64:(h % 2) * 64 + 64, h // 2,
                                 q * 512:(q + 1) * 512], psl)

                # ---- phase 3: partial out = loading @ Wo_hg^T ----
                for i in range(NT):
                    tsl = slice(i * 128, (i + 1) * 128)
                    for n2 in range(2):
                        nsl = slice(n2 * 512, (n2 + 1) * 512)
                        pso = pp3.tile([128, 512], F32, name=f"pso_{it}_{i}_{n2}",
                                       tag="p3")
                        for kc in range(2):
                            nc.tensor.matmul(
                                pso, lt_s[:, kc, tsl], wo_s[:, kc, nsl],
                                start=(kc == 0), stop=(kc == 1))
                        out_s = outp.tile([128, 512], F32, tag="out_s")
                        nc.vector.tensor_copy(out_s, pso)
                        nc.sync.dma_start(out=out_d[tsl, nsl], in_=out_s)

    nc.compile()
    return nc


def host_prep(inputs):
    """Returns per-core in_maps (list of 8 dicts)."""
    states = np.asarray(inputs["states"], np.float32)
    mask = np.asarray(inputs["attention_mask"])
    ln_w = np.asarray(inputs["ln_w"], np.float32)
    time_angles = np.asarray(inputs["time_angles"], np.float32)
    head_time_delta = np.asarray(inputs["head_time_delta"], np.float32)
    Wk = np.asarray(inputs["Wk"], np.float32)
    Wv = np.asarray(inputs["Wv"], np.float32)
    Wo = np.asarray(inputs["Wo"], np.float32)
    for nm in ("bk", "bv", "bo"):
        assert not np.asarray(inputs[nm]).any(), f"{nm} must be zero"

    rnorm = 1.0 / np.sqrt(np.mean(states.astype(np.float64) ** 2, axis=-1) + EPS)
    rnorm = rnorm.astype(np.float32)                     # [B,T]
    scale = (rnorm * mask.astype(np.float32))            # [B,T]

    Wk2 = (Wk * ln_w[None, :]).astype(np.float32)
    Wv2 = (Wv * ln_w[None, :]).astype(np.float32)

    # time embeddings, ang in strict fp32 like the reference
    pos = np.arange(T, dtype=np.float32)[:, None, None]            # [T,1,1]
    pos_q = (pos + head_time_delta[None, :, None]).astype(np.float32)  # [T,H,1]
    ang_q = (pos_q * time_angles).astype(np.float32)               # [T,H,DT]
    cq, sq = np.cos(ang_q), np.sin(ang_q)
    tq = (np.concatenate([cq + sq, cq - sq], -1) * INV_SQRT_DH).astype(np.float32)
    ang_k = (pos[:, 0, :] * time_angles).astype(np.float32)        # [T,DT]
    ck, sk = np.cos(ang_k), np.sin(ang_k)
    tk = (np.concatenate([ck + sk, ck - sk], -1) * INV_SQRT_DH).astype(np.float32)

    sT = [np.ascontiguousarray(states[b].T) for b in range(B)]     # [DIM,T]
    in_maps = []
    for core in range(8):
        b, hg = core // HG, core % HG
        jsl = slice(hg * JG, (hg + 1) * JG)
        hsl = slice(hg * HPG, (hg + 1) * HPG)
        in_maps.append(dict(
            sT=sT[b],
            wkT=np.ascontiguousarray(Wk2[jsl, :].T),
            wvT=np.ascontiguousarray(Wv2[jsl, :].T),
            woT=np.ascontiguousarray(Wo[:, jsl].T),
            tqT=np.ascontiguousarray(
                tq[:, hsl, :].transpose(1, 2, 0).reshape(JG, T)),
            tk=tk,
            sc=np.ascontiguousarray(scale[b]),
            rn=np.ascontiguousarray(rnorm[b]),
        ))
    return in_maps


def gather(results, bo):
    out = np.zeros((B, T, DIM), np.float32)
    for core in range(8):
        out[core // HG] += results[core]["out"]
    if bo.any():
        out += bo[None, None, :]
    return out


_NC_CACHE = {}


def kernel(**inputs) -> np.ndarray:
    if "nc" not in _NC_CACHE:
        _NC_CACHE["nc"] = build_nc()
    nc = _NC_CACHE["nc"]
    in_maps = host_prep(inputs)
    res = bass_utils.run_bass_kernel_spmd(nc, in_maps, core_ids=list(range(8)))
    return gather(res.results, np.asarray(inputs["bo"], np.float32))
